# revision 2
# baseline (speedup 1.0000x reference)
"""Bidirectional 2-layer ConvLSTM (3x3 grid) + FC head — fused SBUF-resident
Trainium2 Bass kernel (v2).

Sharding: data-parallel over batch. B=64 across 8 cores -> 8 batches/core.
Weights replicated; no inter-core communication.

v2 design (vs baseline): no DRAM scratch at all. The layer-l input
projections (conv over x or h0) are computed *into the same PSUM
accumulation region* as the recurrent conv(h) — a single start=True per
PSUM bank plus per-byte pending-zero gives z = conv_in + conv_h directly
in PSUM, with no staging copies and no z DMA. Both time directions run as
independent interleaved chains; x/h0 are streamed from two ends (front
stream for the fwd chain, back stream for the bwd chain) in 16-step
blocks, transposed to channel-major on the PE (spread over the block's
steps). Gates are computed in bf16 (DVE 2x/4x modes), cell state in f32.
Layer outputs accumulate in SBUF (h0 = hf+hb summed in place), layer 1
reads them back as padded blocks, and the FC head contracts straight out
of the summed h1 store.

  A) L0: stream x blocks (2 streams) -> PE transpose -> xpad tiles;
     fused recurrence: xconv(s+1) emitted ahead of hconv(s) so the PE
     never idles on the gate-chain latency.
  B) L1: same loop, input = padded h0 blocks filled by DVE copies.
  C) FC head: 9-tap matmuls from the h1 sum -> out [7, T*BL].
"""

import numpy as np
import ml_dtypes

import concourse.bass as bass
import concourse.mybir as mybir
from concourse.tile import TileContext
from concourse.masks import make_identity

BF16 = mybir.dt.bfloat16
F32 = mybir.dt.float32

B_FULL, T_FULL, C_IN, H, NCLS = 64, 128, 256, 128, 7
NCORES = 8
BL = B_FULL // NCORES  # local batch = 8
TB = 16                # timesteps per stream block
GB = TB * BL           # groups per block = 128

# center tap first: its clipped window covers a gate's full PSUM region, so
# the single start=True (plus per-byte pending-zero) initializes every gate
TAPS = [(1, 1)] + [(dy, dx) for dy in range(3) for dx in range(3) if (dy, dx) != (1, 1)]

SIG = mybir.ActivationFunctionType.Sigmoid
TANH = mybir.ActivationFunctionType.Tanh


def _clip(d):
    # output-pixel range [p0, p0+n) and source range [s0, s0+n) for tap offset d
    if d == 0:
        return 1, 0, 2
    if d == 1:
        return 0, 0, 3
    return 0, 1, 2


def _patch_tile_drain():
    """This walrus rejects >1 sync wait on a Drain: keep the first wait on the
    drain and move the rest onto single-wait NOPs executed just before it."""
    from bass_rust import ScopedClock

    if getattr(TileContext, "_drain_patched", False):
        return

    def _drain_and_barrier(self, tick_clock, wait_clock):
        nc = self.nc
        drain_inst = nc.sync.drain()
        wait_clock.add_sem_waits(
            drain_inst.ins, ScopedClock({None: tick_clock.global_clock})
        )
        si = drain_inst.ins.sync_info
        waits = list(si.on_wait)
        if len(waits) > 1:
            while len(si.on_wait) > 1:
                si.on_wait.pop()
            for w in waits[1:]:
                nop = nc.sync.nop()
                nop.ins.sync_info = mybir.SyncInfo(on_wait=[w], on_update=[])
        nc.all_engine_barrier()
        assert self.sems is not None
        popped = nc._tile_sem_poison_stack.pop()
        assert popped is self._sem_poison
        nc.clear_and_free_semaphores(list(self.sems.allocated().values()))
        nc.all_engine_barrier()

    TileContext._drain_and_barrier = _drain_and_barrier
    TileContext._drain_patched = True


def _fix_multi_waits(raw):
    """This walrus accepts at most 1 sync wait per instruction (2 for
    EventSemaphore). Hoist excess waits onto single-wait EventSemaphore
    carriers inserted just before the instruction on the same engine."""
    import json

    d = json.loads(raw)
    nid = 0
    for fn in d["functions"]:
        for blk in fn["blocks"]:
            out = []
            for inst in blk["instructions"]:
                si = inst.get("sync_info")
                ow = (si or {}).get("on_wait") or []
                cap = 2 if inst.get("opcode") == "EventSemaphore" else 1
                if len(ow) > cap:
                    for w in ow[cap:]:
                        nid += 1
                        out.append({
                            "debug": inst.get("debug", 0),
                            "engine": inst["engine"],
                            "ins": [],
                            "name": f"I-xwait-{nid}",
                            "opcode": "EventSemaphore",
                            "outs": [],
                            "sync_info": {"on_update": [], "on_wait": [w]},
                        })
                    si["on_wait"] = ow[:cap]
                out.append(inst)
            blk["instructions"] = out
    return json.dumps(d).encode()


def build_program(T=T_FULL, has_bias=False, debug=False):
    """Build the per-core Bass program. Returns nc."""
    _patch_tile_drain()
    assert T % (2 * TB) == 0
    NBLK = T // TB
    G = T * BL

    nc = bass.Bass()

    # ---- I/O ----
    x = nc.dram_tensor("x", [BL, T, C_IN, 3, 3], F32, kind="ExternalInput")
    wx0 = {}
    wh0 = {}
    wx1 = {}
    wh1 = {}
    brow_in = {}
    for d in ("f", "b"):
        wx0[d] = nc.dram_tensor(f"wx0{d}", [128, 2, 9, 512], BF16, kind="ExternalInput")
        wh0[d] = nc.dram_tensor(f"wh0{d}", [128, 9, 512], BF16, kind="ExternalInput")
        wx1[d] = nc.dram_tensor(f"wx1{d}", [128, 9, 512], BF16, kind="ExternalInput")
        wh1[d] = nc.dram_tensor(f"wh1{d}", [128, 9, 512], BF16, kind="ExternalInput")
        if has_bias:
            for l in ("0", "1"):
                brow_in[l + d] = nc.dram_tensor(
                    f"brow{l}{d}", [4, 128], BF16, kind="ExternalInput"
                )
    if has_bias:
        ind_in = nc.dram_tensor("ind", [4, 288], BF16, kind="ExternalInput")
    fcw = nc.dram_tensor("fcw", [128, 9, NCLS], BF16, kind="ExternalInput")
    fcb = nc.dram_tensor("fcb", [NCLS, 1], F32, kind="ExternalInput")
    out = nc.dram_tensor("out", [NCLS, G], F32, kind="ExternalOutput")
    if debug:
        h0_dbg = nc.dram_tensor("h0_dbg", [128, T, 3, 3, BL], BF16,
                                kind="ExternalOutput")
        h1_dbg = nc.dram_tensor("h1_dbg", [128, T, 3, 3, BL], BF16,
                                kind="ExternalOutput")

    x_gv = x[:].rearrange("b t c y x -> t b (c y x)")  # (T, BL, 2304)

    def y3(ap):
        return ap.rearrange("p (y x b) -> p y x b", y=3, x=3)

    with TileContext(nc) as tc:
        with tc.tile_pool(name="persist", bufs=1) as pp:
            # summed layer outputs, [ch, t, y, x, b] bf16
            h0st = pp.tile([128, T, 3, 3, BL], BF16, name="h0st", tag="h0st")
            h1st = pp.tile([128, T, 3, 3, BL], BF16, name="h1st", tag="h1st")
            fcw_sb = pp.tile([128, 9, NCLS], BF16, name="fcw", tag="fcw")
            nc.sync.dma_start(out=fcw_sb[:], in_=fcw[:])
            fcb_sb = pp.tile([NCLS, 1], F32, name="fcb", tag="fcb")
            nc.sync.dma_start(out=fcb_sb[:], in_=fcb[:])
            if has_bias:
                ind_sb = pp.tile([4, 288], BF16, name="ind", tag="ind")
                nc.sync.dma_start(out=ind_sb[:], in_=ind_in[:])
                brow_sb = {}
                for key, t_in in brow_in.items():
                    brow_sb[key] = pp.tile([4, 128], BF16, name=f"brow{key}", tag=f"brow{key}")
                    nc.sync.dma_start(out=brow_sb[key][:], in_=t_in[:])

            def zreg(z, gi):
                return y3(z[:, gi * 72 : (gi + 1) * 72])

            def recurrence(layer, zp, gp, xconv, hpad, wh_sb, cst, hst,
                           on_step, on_start):
                """Shared fused recurrence loop, one independent chain per
                time direction (staggered emission so the PE ping-pongs
                between the chains while each one sits in its gate-math
                latency). xconv(s, d, z) emits the step's input projections
                into z (one PSUM bank); on_step(s) emits per-step stream
                maintenance (loads/transposes/fills)."""
                z_tiles = {}

                def emit_xconv(s, d):
                    z = zp.tile([128, 512], F32, name="z", tag=f"z{d}")
                    z_tiles[(s, d)] = z
                    if has_bias:
                        key = f"{layer}{'fb'[d]}"
                        nc.tensor.matmul(
                            z[:, 0:288], brow_sb[key][:], ind_sb[:],
                            start=True, stop=False, skip_group_check=True,
                        )
                    xconv(s, d, z)

                def emit_hconv(s, d):
                    z = z_tiles[(s, d)]
                    hp_r = hpad[d][s % 2]
                    for gi in range(4):
                        dst = zreg(z, gi)
                        for k, (dy, dx) in enumerate(TAPS):
                            py, sy, ny = _clip(dy)
                            px, sx, nx = _clip(dx)
                            nc.tensor.matmul(
                                dst[:, py : py + ny, px : px + nx, :],
                                wh_sb["fb"[d]][:, dy * 3 + dx,
                                               gi * 128 : (gi + 1) * 128],
                                hp_r[:, 1 + sy : 1 + sy + ny,
                                     1 + sx : 1 + sx + nx, :],
                                start=False,
                                stop=(gi == 3 and k == 8),
                                skip_group_check=True,
                            )

                def emit_front(s, d):
                    # gates: sigmoid(i,f,o) + tanh(g) straight from PSUM
                    z = z_tiles.pop((s, d))
                    g = gp.tile([128, 4, 72], BF16, name="g", tag=f"g{d}")
                    nc.scalar.activation(
                        g[:, 0:3, :],
                        z[:, 0:216].rearrange("p (q c) -> p q c", q=3),
                        SIG,
                    )
                    nc.scalar.activation(g[:, 3, :], z[:, 216:288], TANH)
                    ig = gp.tile([128, 72], BF16, name="ig", tag=f"ig{d}")
                    nc.vector.tensor_mul(ig[:], g[:, 0, :], g[:, 3, :])
                    cf = gp.tile([128, 72], F32, name="cf", tag=f"cf{d}")
                    nc.vector.tensor_mul(cf[:], g[:, 1, :], cst[d][:])
                    nc.vector.tensor_add(cst[d][:], ig[:], cf[:])
                    return g

                def emit_back(s, d, g):
                    tcl = gp.tile([128, 72], BF16, name="tc", tag=f"tc{d}")
                    nc.scalar.activation(tcl[:], cst[d][:], TANH)
                    t_d = s if d == 0 else T - 1 - s
                    so3 = y3(g[:, 2, :])
                    tc3 = y3(tcl[:])
                    # hpad write is on every step's critical chain: Pool is
                    # empty, so it never queues behind other gate math
                    nc.gpsimd.tensor_mul(
                        hpad[d][(s + 1) % 2][:, 1:4, 1:4, :], so3, tc3
                    )
                    if s < T // 2:
                        # first writer of slot t_d
                        nc.vector.tensor_mul(hst[:, t_d], so3, tc3)
                    else:
                        ht = gp.tile([128, 72], BF16, name=f"ht{d}",
                                     tag=f"ht{d}")
                        nc.vector.tensor_mul(ht[:], so3, tc3)
                        nc.vector.tensor_add(hst[:, t_d], hst[:, t_d],
                                             y3(ht[:]))

                # software-pipelined schedule: between chain d's hconv(s) and
                # hconv(s+1), the PE runs the other chain's hconv plus two
                # steps' worth of x-projections, covering the gate latency
                on_start()
                emit_xconv(0, 0)
                emit_xconv(1, 0)
                emit_xconv(0, 1)
                for s in range(T):
                    emit_hconv(s, 0)
                    gf = emit_front(s, 0)
                    if s + 1 < T:
                        emit_xconv(s + 1, 1)
                    emit_hconv(s, 1)
                    gb = emit_front(s, 1)
                    on_step(s)
                    if s + 2 < T:
                        emit_xconv(s + 2, 0)
                    emit_back(s, 0, gf)
                    emit_back(s, 1, gb)

            # ================= Layer 0 =================
            with (
                tc.tile_pool(name="l0w", bufs=1) as wp,
                tc.tile_pool(name="l0xg", bufs=1) as xgp,
                tc.tile_pool(name="l0xbf", bufs=2) as xbfp,
                tc.tile_pool(name="l0st", bufs=1) as stp,
                tc.tile_pool(name="l0z", bufs=4, space="PSUM") as zp,
                tc.tile_pool(name="l0g", bufs=3) as gp,
            ):
                wx0_sb = {}
                wh0_sb = {}
                for d in ("f", "b"):
                    wx0_sb[d] = wp.tile([128, 2, 9, 512], BF16, name=f"wx0{d}", tag=f"wx0{d}")
                    nc.sync.dma_start(out=wx0_sb[d][:], in_=wx0[d][:])
                    wh0_sb[d] = wp.tile([128, 9, 512], BF16, name=f"wh0{d}", tag=f"wh0{d}")
                    nc.sync.dma_start(out=wh0_sb[d][:], in_=wh0[d][:])
                xpad = {
                    S: [stp.tile([128, 2, 5, 5, GB], BF16, name=f"xp{S}{p}",
                                 tag=f"xp{S}{p}") for p in range(2)]
                    for S in "FB"
                }
                for S in "FB":
                    nc.gpsimd.memset(xpad[S][0][:], 0.0)
                    nc.gpsimd.memset(xpad[S][1][:], 0.0)
                hpad0 = {}
                for d in range(2):
                    hpad0[d] = [stp.tile([128, 5, 5, BL], BF16,
                                         name=f"h0p{d}{p}", tag=f"h0p{d}{p}")
                                for p in range(2)]
                    nc.gpsimd.memset(hpad0[d][0][:], 0.0)
                    nc.gpsimd.memset(hpad0[d][1][:], 0.0)
                cst0 = {}
                for d in range(2):
                    cst0[d] = stp.tile([128, 72], F32, name=f"c0{d}",
                                       tag=f"c0{d}")
                    nc.gpsimd.memset(cst0[d][:], 0.0)

                cur_xg = {}
                cur_xbf = {}

                def load_block(S, blk):
                    t0 = blk * TB if S == "F" else T - (blk + 1) * TB
                    xgt = xgp.tile([GB, C_IN * 9], F32, name=f"xg{S}", tag=f"xg{S}")
                    for ts in range(TB):
                        nc.sync.dma_start(
                            out=xgt[ts * BL : (ts + 1) * BL, :],
                            in_=x_gv[t0 + ts],
                        )
                    cur_xg[(S, blk)] = xgt

                def convert_block(S, blk):
                    # f32 -> bf16, reordering (c yx) -> (yx c) so each xbar
                    # transpose reads a contiguous [g, 128c] slab
                    src = cur_xg.pop((S, blk))
                    xbt = xbfp.tile([GB, C_IN * 9], BF16, name=f"xbf{S}",
                                    tag=f"xbf{S}")
                    nc.scalar.copy(
                        xbt[:].rearrange("g (yx c) -> g c yx", c=C_IN),
                        src[:].rearrange("g (c yx) -> g c yx", yx=9),
                    )
                    cur_xbf[(S, blk)] = xbt

                def emit_tr(S, blk, j):
                    # transpose [g, ci] -> xpad[ci, ..g] on the DMA xbar
                    cb, yx = divmod(j, 9)
                    xbv = cur_xbf[(S, blk)][:].rearrange(
                        "g (yx c) -> g yx c", c=C_IN
                    )
                    nc.sync.dma_start_transpose(
                        out=xpad[S][blk % 2][:, cb, 1 + yx // 3, 1 + yx % 3, :],
                        in_=xbv[:, yx, cb * 128 : (cb + 1) * 128],
                    )

                def xconv0(s, d, z):
                    blk, pos = divmod(s, TB)
                    S = "F" if d == 0 else "B"
                    g0 = pos * BL if d == 0 else (TB - 1 - pos) * BL
                    src = xpad[S][blk % 2]
                    for gi in range(4):
                        dst = zreg(z, gi)
                        k = 0
                        for dy, dx in TAPS:
                            py, sy, ny = _clip(dy)
                            px, sx, nx = _clip(dx)
                            for cb in range(2):
                                nc.tensor.matmul(
                                    dst[:, py : py + ny, px : px + nx, :],
                                    wx0_sb["fb"[d]][
                                        :, cb, dy * 3 + dx,
                                        gi * 128 : (gi + 1) * 128],
                                    src[:, cb, 1 + sy : 1 + sy + ny,
                                        1 + sx : 1 + sx + nx,
                                        g0 : g0 + BL],
                                    start=(not has_bias and gi == 0
                                           and k == 0),
                                    stop=False,
                                    skip_group_check=True,
                                )
                                k += 1

                def on_start0():
                    # stage block 0 before any xconv is emitted (the tile
                    # framework orders by emission: readers emitted first
                    # would see the memset state), then start block-1 loads
                    load_block("F", 0)
                    convert_block("F", 0)
                    for j in range(18):
                        emit_tr("F", 0, j)
                    load_block("B", 0)
                    convert_block("B", 0)
                    for j in range(18):
                        emit_tr("B", 0, j)
                    load_block("F", 1)
                    load_block("B", 1)

                def on_step0(s):
                    # during block blk: xbar-transpose block blk+1 (loaded one
                    # block earlier, converted to bf16 at pos 0) and DMA-load
                    # block blk+2
                    blk, pos = divmod(s, TB)
                    if pos == 0:
                        if blk + 1 < NBLK:
                            convert_block("F", blk + 1)
                            convert_block("B", blk + 1)
                        if blk + 2 < NBLK:
                            load_block("F", blk + 2)
                            load_block("B", blk + 2)
                    if blk + 1 < NBLK and 2 <= pos < 14:
                        for j in range(3 * (pos - 2), 3 * (pos - 2) + 3):
                            S, jj = ("F", j) if j < 18 else ("B", j - 18)
                            emit_tr(S, blk + 1, jj)

                recurrence("0", zp, gp, xconv0, hpad0, wh0_sb, cst0, h0st,
                           on_step0, on_start0)

            # ================= Layer 1 =================
            with (
                tc.tile_pool(name="l1w", bufs=1) as wp1,
                tc.tile_pool(name="l1st", bufs=1) as stp1,
                tc.tile_pool(name="l1z", bufs=4, space="PSUM") as zp1,
                tc.tile_pool(name="l1g", bufs=3) as gp1,
            ):
                wx1_sb = {}
                wh1_sb = {}
                for d in ("f", "b"):
                    wx1_sb[d] = wp1.tile([128, 9, 512], BF16, name=f"wx1{d}", tag=f"wx1{d}")
                    nc.sync.dma_start(out=wx1_sb[d][:], in_=wx1[d][:])
                    wh1_sb[d] = wp1.tile([128, 9, 512], BF16, name=f"wh1{d}", tag=f"wh1{d}")
                    nc.sync.dma_start(out=wh1_sb[d][:], in_=wh1[d][:])
                h0pad = {
                    S: [stp1.tile([128, 5, 5, GB], BF16, name=f"hp{S}{p}",
                                  tag=f"hp{S}{p}") for p in range(2)]
                    for S in "FB"
                }
                for S in "FB":
                    nc.gpsimd.memset(h0pad[S][0][:], 0.0)
                    nc.gpsimd.memset(h0pad[S][1][:], 0.0)
                hpad1 = {}
                for d in range(2):
                    hpad1[d] = [stp1.tile([128, 5, 5, BL], BF16,
                                          name=f"h1p{d}{p}", tag=f"h1p{d}{p}")
                                for p in range(2)]
                    nc.gpsimd.memset(hpad1[d][0][:], 0.0)
                    nc.gpsimd.memset(hpad1[d][1][:], 0.0)
                cst1 = {}
                for d in range(2):
                    cst1[d] = stp1.tile([128, 72], F32, name=f"c1{d}",
                                        tag=f"c1{d}")
                    nc.gpsimd.memset(cst1[d][:], 0.0)

                def fill_h0(S, blk):
                    t0 = blk * TB if S == "F" else T - (blk + 1) * TB
                    nc.vector.tensor_copy(
                        h0pad[S][blk % 2][:, 1:4, 1:4, :].rearrange(
                            "p y x (t b) -> p y x t b", t=TB
                        ),
                        h0st[:, t0 : t0 + TB].rearrange(
                            "p t y x b -> p y x t b"
                        ),
                    )

                def xconv1(s, d, z):
                    blk, pos = divmod(s, TB)
                    S = "F" if d == 0 else "B"
                    g0 = pos * BL if d == 0 else (TB - 1 - pos) * BL
                    src = h0pad[S][blk % 2]
                    for gi in range(4):
                        dst = zreg(z, gi)
                        for k, (dy, dx) in enumerate(TAPS):
                            py, sy, ny = _clip(dy)
                            px, sx, nx = _clip(dx)
                            nc.tensor.matmul(
                                dst[:, py : py + ny, px : px + nx, :],
                                wx1_sb["fb"[d]][
                                    :, dy * 3 + dx,
                                    gi * 128 : (gi + 1) * 128],
                                src[:, 1 + sy : 1 + sy + ny,
                                    1 + sx : 1 + sx + nx, g0 : g0 + BL],
                                start=(not has_bias and gi == 0
                                       and k == 0),
                                stop=False,
                                skip_group_check=True,
                            )

                def on_start1():
                    fill_h0("F", 0)
                    fill_h0("B", 0)

                def on_step1(s):
                    blk, pos = divmod(s, TB)
                    if pos == 0 and blk + 1 < NBLK:
                        fill_h0("F", blk + 1)
                        fill_h0("B", blk + 1)

                recurrence("1", zp1, gp1, xconv1, hpad1, wh1_sb, cst1, h1st,
                           on_step1, on_start1)

            # ================= FC head =================
            with (
                tc.tile_pool(name="fc", bufs=2) as fp,
                tc.tile_pool(name="fcp", bufs=2, space="PSUM") as fpp,
            ):
                for blk in range(NBLK):
                    t0 = blk * TB
                    h1v = h1st[:, t0 : t0 + TB]  # [128, TB, 3, 3, BL]
                    ps = fpp.tile([NCLS, GB], F32, name="ps", tag="ps")
                    psv = ps[:].rearrange("p (t b) -> p t b", t=TB)
                    for k, (dy, dx) in enumerate(TAPS):
                        nc.tensor.matmul(
                            psv,
                            fcw_sb[:, dy * 3 + dx, :],
                            h1v[:, :, dy, dx, :],
                            start=(k == 0),
                            stop=(k == 8),
                        )
                    ot = fp.tile([NCLS, GB], F32, name="ot", tag="ot")
                    nc.vector.tensor_scalar_add(ot[:], ps[:], fcb_sb[:, 0:1])
                    nc.sync.dma_start(
                        out=out[:, blk * GB : (blk + 1) * GB], in_=ot[:]
                    )
                if debug:
                    nc.sync.dma_start(out=h0_dbg[:], in_=h0st[:])
                    nc.sync.dma_start(out=h1_dbg[:], in_=h1st[:])

    _orig_to_json = nc.to_json_bytes
    nc.to_json_bytes = lambda: _fix_multi_waits(_orig_to_json())
    return nc


# ---------------- host side ----------------

def _prep_weights(w, b, cin):
    """w: (512, cin+128, 3, 3) -> (wx, wh) bf16 host arrays + brow (4,128)."""
    bf = ml_dtypes.bfloat16
    wx = w[:, :cin].reshape(512, cin, 9)            # (co, ci, tap)
    wx = wx.transpose(1, 2, 0)                      # (ci, tap, co)
    if cin == 256:
        wx = wx.reshape(2, 128, 9, 512).transpose(1, 0, 2, 3)  # (128, 2, 9, 512)
    wx = np.ascontiguousarray(wx).astype(bf)
    wh = w[:, cin:].reshape(512, 128, 9).transpose(1, 2, 0)    # (128, 9, 512)
    wh = np.ascontiguousarray(wh).astype(bf)
    brow = np.ascontiguousarray(b.reshape(4, 128)).astype(bf)
    return wx, wh, brow


def make_inputs_core(core, has_bias, x, w_f0, b_f0, w_b0, b_b0, w_f1, b_f1,
                     w_b1, b_b1, fc_w, fc_b):
    m = {"x": np.ascontiguousarray(x[core * BL : (core + 1) * BL])}
    for d, w, b in (("f", w_f0, b_f0), ("b", w_b0, b_b0)):
        wx, wh, brow = _prep_weights(np.asarray(w), np.asarray(b), 256)
        m[f"wx0{d}"], m[f"wh0{d}"] = wx, wh
        if has_bias:
            m[f"brow0{d}"] = brow
    for d, w, b in (("f", w_f1, b_f1), ("b", w_b1, b_b1)):
        wx, wh, brow = _prep_weights(np.asarray(w), np.asarray(b), 128)
        m[f"wx1{d}"], m[f"wh1{d}"] = wx, wh
        if has_bias:
            m[f"brow1{d}"] = brow
    if has_bias:
        ind = np.zeros((4, 288), np.float32)
        for g in range(4):
            ind[g, g * 72 : (g + 1) * 72] = 1.0
        m["ind"] = ind.astype(ml_dtypes.bfloat16)
    fcw = np.asarray(fc_w).reshape(NCLS, 128, 9).transpose(1, 2, 0)  # (128, 9, 7)
    m["fcw"] = np.ascontiguousarray(fcw).astype(ml_dtypes.bfloat16)
    m["fcb"] = np.ascontiguousarray(np.asarray(fc_b).reshape(NCLS, 1)).astype(np.float32)
    return m


_nc_cache = {}


def kernel(**inputs):
    from concourse.bass_utils import run_bass_kernel_spmd

    has_bias = any(
        np.any(np.asarray(inputs[k]))
        for k in ("b_f0", "b_b0", "b_f1", "b_b1")
    )
    key = ("nc", has_bias)
    if key not in _nc_cache:
        _nc_cache[key] = build_program(T_FULL, has_bias=has_bias)
        _nc_cache["nc"] = _nc_cache[key]
    nc = _nc_cache[key]
    x = np.asarray(inputs["x"], dtype=np.float32)
    in_maps = [make_inputs_core(c, has_bias, x, inputs["w_f0"], inputs["b_f0"],
                                inputs["w_b0"], inputs["b_b0"],
                                inputs["w_f1"], inputs["b_f1"],
                                inputs["w_b1"], inputs["b_b1"],
                                inputs["fc_w"], inputs["fc_b"])
               for c in range(NCORES)]
    res = run_bass_kernel_spmd(nc, in_maps, core_ids=list(range(NCORES)))
    outs = []
    for c in range(NCORES):
        o = res.results[c]["out"]  # (7, G) with g = t*BL + b
        o = o.reshape(NCLS, T_FULL, BL).transpose(2, 1, 0)  # (BL, T, 7)
        outs.append(o)
    return np.ascontiguousarray(np.concatenate(outs, axis=0), dtype=np.float32)
